# revision 5
# baseline (speedup 1.0000x reference)
"""AFT-Full attention block on 8 TRN2 NeuronCores.

Data-parallel over (batch x spatial): the computation is independent per
spatial column, so the 4*4096 = 16384 columns are split into 8 shards of
2048 columns (one per core). All params are tiny and replicated.

Per-core Bass kernel (Tile framework), paired-partition layout: two
512-column chunks (A, B) are processed together with chunk A living on
partitions 0-63 and chunk B on partitions 64-127, so the IC=64-row
elementwise stage uses all 128 partitions.  fp32r matmuls cannot write
PSUM partitions 64-127 directly, so the pair tiles are built with
accumulating matmul pairs: [W|0] @ xa (start) then [0|W] @ xb (accum).
"""

import os
import sys

sys.path.insert(0, "/opt/trn_rl_repo")

import numpy as np

from concourse import bacc, mybir, tile
from concourse.bass_utils import run_bass_kernel_spmd

BS, C, HH, WW = 4, 128, 64, 64
T = HH * WW          # 4096 spatial positions per batch elem
IC = C // 2          # 64
NCORES = 8
NCOL = BS * T // NCORES   # 2048 columns per core
F = 512                   # chunk size (columns); one PSUM bank of f32
PAIR = 2 * F              # columns per A/B pair
NPAIR = NCOL // PAIR      # pairs per core
WCOLS = 704               # packed weight tile columns

_f32 = mybir.dt.float32
_f32r = mybir.dt.float32r

_cached = {}


def _install_ntff_hook():
    """Register the axon NTFF profiling hook (the agent image's antenv
    lacks axon_hooks, so concourse can't find it; recreate it here)."""
    import types

    if "antenv.axon_hooks" in sys.modules:
        return
    mod = types.ModuleType("antenv.axon_hooks")
    state = {"hook": None}
    mod.set_axon_ntff_profile_hook = lambda h: state.update(hook=h)
    mod.get_axon_ntff_profile_hook = lambda: state["hook"]
    sys.modules["antenv.axon_hooks"] = mod
    try:
        sys.path.insert(0, "/root/.axon_site")
        from trn_agent_boot.trn_boot import _ntff_profile_via_ctypes

        hook = _ntff_profile_via_ctypes("/opt/axon/libaxon_pjrt.so")
        if hook is not None:
            mod.set_axon_ntff_profile_hook(hook)
    except Exception as e:  # degrade to no tracing
        print(f"ntff hook install failed: {e}", file=sys.stderr)


def _build():
    nc = bacc.Bacc("TRN2", target_bir_lowering=False, debug=False)
    x_ext = nc.dram_tensor("x", [C, NCOL], _f32r, kind="ExternalInput")
    w_ext = nc.dram_tensor("w", [C, WCOLS], _f32r, kind="ExternalInput")
    out_ext = nc.dram_tensor("out", [C, NCOL], _f32, kind="ExternalOutput")

    EXP = mybir.ActivationFunctionType.Exp

    with tile.TileContext(nc) as tc:
        with (
            tc.tile_pool(name="wpool", bufs=1) as wpool,
            tc.tile_pool(name="xpool", bufs=4) as xpool,
            tc.tile_pool(name="sb", bufs=2) as sb,
            tc.tile_pool(name="osb", bufs=2) as osb,
            tc.tile_pool(name="psum", bufs=1, space="PSUM") as ps,
        ):
            w = wpool.tile([C, WCOLS], _f32r)
            nc.sync.dma_start(w[:], w_ext[:])
            # packed layout: [Z WkT Z WqT Z WvT Z | eB-blockdiag | WmT x2]
            w_k0, w_k1 = w[:, 64:192], w[:, 0:128]
            w_q0, w_q1 = w[:, 192:320], w[:, 128:256]
            w_v0, w_v1 = w[:, 320:448], w[:, 256:384]
            w_eB = w[:, 448:576]
            w_m = w[:, 576:704]

            for p in range(NPAIR):
                base = p * PAIR
                xa = xpool.tile([C, F], _f32r, tag="x")
                nc.sync.dma_start(xa[:], x_ext[:, base:base + F])
                xb = xpool.tile([C, F], _f32r, tag="x")
                nc.sync.dma_start(xb[:], x_ext[:, base + F:base + PAIR])

                # pair tiles {A;B} built by accumulating matmul pairs
                pk = ps.tile([C, F], _f32, tag="pk")
                nc.tensor.matmul(pk[:], w_k0, xa[:], start=True, stop=False)
                nc.tensor.matmul(pk[:], w_k1, xb[:], start=False, stop=True)
                pq = ps.tile([C, F], _f32, tag="pq")
                nc.tensor.matmul(pq[:], w_q0, xa[:], start=True, stop=False)
                nc.tensor.matmul(pq[:], w_q1, xb[:], start=False, stop=True)
                pv = ps.tile([C, F], _f32, tag="pv")
                nc.tensor.matmul(pv[:], w_v0, xa[:], start=True, stop=False)
                nc.tensor.matmul(pv[:], w_v1, xb[:], start=False, stop=True)

                ek = sb.tile([C, F], _f32r, tag="ek")
                nc.scalar.activation(ek[:], pk[:], EXP)
                ekv = sb.tile([C, F], _f32r, tag="ekv")
                nc.vector.tensor_mul(ekv[:], ek[:], pv[:])

                # den/num for both chunks via block-diagonal [eB.T, eB.T]
                pden = ps.tile([C, F], _f32, tag="pden")
                nc.tensor.matmul(pden[:], w_eB, ek[:])
                pnum = ps.tile([C, F], _f32, tag="pnum")
                nc.tensor.matmul(pnum[:], w_eB, ekv[:])

                rden = sb.tile([C, F], _f32, tag="rden")
                nc.vector.reciprocal_approx_fast(rden[:], pden[:])
                r = sb.tile([C, F], _f32, tag="r")
                nc.vector.tensor_mul(r[:], rden[:], pnum[:])
                o1 = sb.tile([C, F], _f32r, tag="o1")
                nc.vector.tensor_mul(o1[:], r[:], pq[:])

                po_a = ps.tile([C, F], _f32, tag="poa")
                nc.tensor.matmul(po_a[:], w_m[0:64, :], o1[0:64, :])
                po_b = ps.tile([C, F], _f32, tag="pob")
                nc.tensor.matmul(po_b[:], w_m[64:128, :], o1[64:128, :])

                ot = osb.tile([C, PAIR], _f32, tag="ot")
                nc.scalar.copy(ot[:, 0:F], po_a[:])
                nc.scalar.copy(ot[:, F:PAIR], po_b[:])
                nc.scalar.dma_start(out_ext[:, base:base + PAIR], ot[:])

    nc.compile()
    return nc


def _pack_weights(Wq, Wk, Wv, B, Wm):
    eB = np.exp(B)
    blk = np.zeros((C, C), np.float32)
    blk[0:IC, 0:IC] = eB.T
    blk[IC:C, IC:C] = eB.T
    wcat = np.zeros((C, WCOLS), np.float32)
    wcat[:, 64:128] = Wk.T
    wcat[:, 192:256] = Wq.T
    wcat[:, 320:384] = Wv.T
    wcat[:, 448:576] = blk
    wcat[0:IC, 576:704] = Wm.T
    wcat[IC:C, 576:704] = Wm.T
    return wcat


def kernel(x, Wq, Wk, Wv, B, Wm):
    x = np.ascontiguousarray(np.asarray(x, dtype=np.float32))
    Wq = np.asarray(Wq, dtype=np.float32)
    Wk = np.asarray(Wk, dtype=np.float32)
    Wv = np.asarray(Wv, dtype=np.float32)
    B = np.asarray(B, dtype=np.float32)
    Wm = np.asarray(Wm, dtype=np.float32)

    xf = x.reshape(BS, C, T)
    per_batch = NCORES // BS  # shards per batch elem
    shards = []
    for core in range(NCORES):
        b, j = divmod(core, per_batch)
        shards.append(np.ascontiguousarray(xf[b, :, j * NCOL:(j + 1) * NCOL]))

    wcat = _pack_weights(Wq, Wk, Wv, B, Wm)

    if "nc" not in _cached:
        _cached["nc"] = _build()
    nc = _cached["nc"]

    in_maps = [{"x": shards[i], "w": wcat} for i in range(NCORES)]
    trace = bool(int(os.environ.get("AFT_TRACE", "0")))
    if trace:
        _install_ntff_hook()
    res = run_bass_kernel_spmd(
        nc, in_maps, core_ids=list(range(NCORES)), trace=trace
    )
    kernel.last_exec_time_ns = res.exec_time_ns
    kernel.last_results = res

    out = np.empty((BS, C, T), np.float32)
    for core in range(NCORES):
        b, j = divmod(core, per_batch)
        out[b, :, j * NCOL:(j + 1) * NCOL] = res.results[core]["out"]
    return out.reshape(BS, C, HH, WW)


kernel.last_exec_time_ns = None
kernel.last_results = None
